# revision 23
# baseline (speedup 1.0000x reference)
"""Causal self-attention (B=2, T=2048, D=2048, 16 heads) on 8 trn2 cores.

Sharding: tensor-parallel over heads - 2 heads per core. Each core computes
q/k/v projections for its 2 heads (column-parallel), causal attention per
head, and a partial output projection (row-parallel). Host sums the 8
partial outputs.

v2 design (vs baseline): all-bf16 data path, single x pass with both heads
fused, v projected x-stationary directly into [token, d] layout (no PE
transposes), batch-pipelined emission so attention b0 overlaps proj b1 and
out-proj b0 overlaps attention b1, exact 8-bank PSUM layout, copies split
across DVE/ACT, few large DMAs.

Layouts (per core):
  xT4   [128, tch(8), kt(16), 512]   host-packed x, contiguous per DMA
  wq/wk [2h, 128(in-sub), kt, 128]   lhsT for W-stationary q/k projections
  wv    [2h, 128(in-sub), kt, 128]   rhs for x-stationary v projection
  wo    [2h, 128(d), 2048]           rhs (moving) for out-proj
  qT/kT [128(d), 2048] per (h, b)    head dim on partitions
  v     [128(tok), 16, 128] per (h,b) token tiles on partitions
  outT  [128(d), 2048] per (h, b)
  y     [4096, 2048]                 partial output (summed on host)
"""

import math
from contextlib import ExitStack

import numpy as np
import ml_dtypes

import concourse.bass as bass
import concourse.mybir as mybir
import concourse.tile as tile
from concourse import bacc
from concourse.bass_utils import run_bass_kernel_spmd

P = 128
DM = 2048          # d_model
B, T = 2, 2048
BT = B * T         # 4096
NCORES = 8
HPC = 2            # heads per core
D = 128            # head dim
KD = DM // P       # 16 contraction tiles
IC = 512           # query chunk width
NI = T // IC       # 4 query chunks per batch
TCH = BT // IC     # 8 token chunks
TJ = T // P        # 16 key tiles per batch

F32 = mybir.dt.float32
BF16 = mybir.dt.bfloat16
F32R = mybir.dt.float32r
FP8 = mybir.dt.float8e4
_NP = {BF16: ml_dtypes.bfloat16, F32: np.float32, F32R: np.float32,
       FP8: ml_dtypes.float8_e4m3}

# s: qT/kT storage (S matmul inputs); o: outT storage (outproj lhsT);
# y: partial-output DMA dtype; p8: fp8+DoubleRow q/k projections.
CFG_FAST = dict(s=BF16, o=BF16, y=BF16, p8=False)
CFG_BF16 = dict(s=BF16, o=BF16, y=BF16, p8=False)
CFG_ACC = dict(s=F32R, o=F32R, y=F32, p8=False)
DEBUG_DUMPS = False


def _emit(tc, cfg, xT4, xT8, wq, wk, wv, wo, y, dbg=None):
    nc = tc.nc
    s_dt = cfg["s"]
    o_dt = cfg["o"]
    p8 = cfg["p8"]
    qk_dt = FP8 if p8 else BF16
    scale = 1.0 / math.sqrt(D)

    with ExitStack() as ctx:
        consts = ctx.enter_context(tc.tile_pool(name="consts", bufs=1))
        wpool = ctx.enter_context(tc.tile_pool(name="wpool", bufs=1))
        xpool = ctx.enter_context(tc.tile_pool(name="xpool", bufs=2))
        arrs = ctx.enter_context(tc.tile_pool(name="arrs", bufs=1))
        ptpool = ctx.enter_context(tc.tile_pool(name="ptpool", bufs=6))
        accpool = ctx.enter_context(tc.tile_pool(name="accpool", bufs=3))
        smalls = ctx.enter_context(tc.tile_pool(name="smalls", bufs=3))
        ypool = ctx.enter_context(tc.tile_pool(name="ypool", bufs=4))
        psum = ctx.enter_context(tc.tile_pool(name="psum", bufs=1, space="PSUM"))

        # ---- constants ----
        ones_col = consts.tile([P, 1], BF16, tag="ones", name="ones")
        nc.vector.memset(ones_col, 1.0)

        # tri[p, i] = 1.0 if i >= p else 0 (keep lower-triangular in S.T)
        tri = consts.tile([P, P], BF16, tag="tri", name="tri")
        nc.gpsimd.memset(tri, 0.0)
        nc.gpsimd.affine_select(
            out=tri, in_=tri, compare_op=mybir.AluOpType.is_gt,
            fill=1.0, base=0, pattern=[[-1, P]], channel_multiplier=1,
        )

        # ---- warmup: dense dummy matmuls warm the PE clock (HAM) while the
        # first DMAs are in flight; they retire before real work is ready.
        warm = consts.tile([P, IC], BF16, tag="warm", name="warm")
        nc.vector.memset(warm, 0.0)
        ps_w = psum.tile([P, IC], F32, tag="o0", name="warmps")
        for _ in range(20):
            nc.tensor.matmul(ps_w, warm[:, :P], warm, start=True, stop=True,
                             skip_group_check=True)

        # ---- persistent weights (tiles now; DMAs issued in emission order
        # below so x/h0 weights land first) ----
        wq_sb, wk_sb, wv_sb, wo_sb = [], [], [], []
        for h in range(HPC):
            for lst, nm, dt in ((wq_sb, "wq", qk_dt), (wk_sb, "wk", qk_dt),
                                (wv_sb, "wv", BF16)):
                lst.append(wpool.tile([P, KD, P], dt, tag=f"{nm}{h}",
                                      name=f"{nm}{h}"))
            wo_sb.append(wpool.tile([P, DM], BF16, tag=f"wo{h}",
                                    name=f"wo{h}"))

        def load_w(h):
            for t, src in ((wq_sb[h], wq), (wk_sb[h], wk), (wv_sb[h], wv)):
                nc.sync.dma_start(t, src[h])


        # ---- per-(head, batch) arrays ----
        qT = [[arrs.tile([P, T], s_dt, tag=f"qT{h}{b}", name=f"qT{h}{b}")
               for b in range(B)] for h in range(HPC)]
        kT = [[arrs.tile([P, T], s_dt, tag=f"kT{h}{b}", name=f"kT{h}{b}")
               for b in range(B)] for h in range(HPC)]
        v_sb = [[arrs.tile([P, TJ, D], BF16, tag=f"v{h}{b}", name=f"v{h}{b}")
                 for b in range(B)] for h in range(HPC)]
        outT = [[arrs.tile([P, T], o_dt, tag=f"oT{h}{b}", name=f"oT{h}{b}")
                 for b in range(B)] for h in range(HPC)]

        eng = [0]  # alternating copy-engine counter

        def copy_out(dst, src):
            if eng[0] % 2 == 0:
                nc.vector.tensor_copy(dst, src)
            else:
                nc.scalar.copy(dst, src)
            eng[0] += 1

        # ---- QKV projection for one 512-token chunk (both heads) ----
        def proj_tch(tch, xt=None, xt8=None):
            b, tc4 = tch // 4, tch % 4
            tsl = slice(tc4 * IC, (tc4 + 1) * IC)
            if xt is None:
                xt = xpool.tile([P, KD, IC], BF16, tag="xt", name="xt")
                nc.sync.dma_start(xt, xT4[:, tch])
                if p8:
                    xt8 = xpool.tile([P, KD, IC], FP8, tag="xt8", name="xt8")
                    nc.sync.dma_start(xt8, xT8[:, tch])
            for h in range(HPC):
                psq = psum.tile([P, IC], F32, tag="pq", name="pq")
                psk = psum.tile([P, IC], F32, tag="pk", name="pk")
                psv = psum.tile([P, IC], F32, tag="pv", name="pv")
                if p8:
                    # fp8 DoubleRow: contract two 128-tiles per pass
                    for k2 in range(KD // 2):
                        st, sp = k2 == 0, k2 == KD // 2 - 1
                        ksl = slice(2 * k2, 2 * k2 + 2)
                        nc.tensor.matmul(
                            psq, wq_sb[h][:, ksl], xt8[:, ksl],
                            start=st, stop=sp,
                            perf_mode=mybir.MatmulPerfMode.DoubleRow)
                        nc.tensor.matmul(
                            psk, wk_sb[h][:, ksl], xt8[:, ksl],
                            start=st, stop=sp,
                            perf_mode=mybir.MatmulPerfMode.DoubleRow)
                else:
                    for kt in range(KD):
                        st, sp = kt == 0, kt == KD - 1
                        nc.tensor.matmul(psq, wq_sb[h][:, kt], xt[:, kt],
                                         start=st, stop=sp)
                        nc.tensor.matmul(psk, wk_sb[h][:, kt], xt[:, kt],
                                         start=st, stop=sp)
                for kt in range(KD):
                    st, sp = kt == 0, kt == KD - 1
                    for sub in range(4):
                        # start=True clears has_written for the whole bank, so
                        # only the bank's first matmul may set it; the other
                        # kt=0 sub-tiles write via their cleared bits.
                        nc.tensor.matmul(
                            psv[:, sub * D:(sub + 1) * D],
                            xt[:, kt, sub * P:(sub + 1) * P], wv_sb[h][:, kt],
                            start=(st and sub == 0), stop=(sp and sub == 3),
                            skip_group_check=True)
                copy_out(qT[h][b][:, tsl], psq)
                copy_out(kT[h][b][:, tsl], psk)
                copy_out(v_sb[h][b][:, tc4 * 4:(tc4 + 1) * 4], psv)

        # ---- attention for one (head, batch, 512-query chunk) ----
        cseq = [0]  # emission-order chunk counter for psum rotation

        def attn_chunk(h, b, ic, s3=False):
            nj = 4 * (ic + 1)
            nfull = 4 * ic
            qoff = ic * IC
            ck = cseq[0]
            cseq[0] += 1
            stags = ("s0", "s1", "pv") if s3 else ("s0", "s1")
            qs = qT[h][b][:, qoff:qoff + IC]
            ps_o = psum.tile([P, IC], F32, tag=f"o{ck % 2}", name=f"o{ck % 2}")
            pt_acc = accpool.tile([P, IC], BF16, tag="ptacc", name="ptacc")
            for jt in range(nj):
                m = jt - ic * 4
                lo = max(m, 0) * P
                ps_s = psum.tile([P, IC], F32, tag=stags[jt % len(stags)],
                                 name="ps_s")
                nc.tensor.matmul(
                    ps_s[:, lo:], kT[h][b][:, jt * P:(jt + 1) * P],
                    qs[:, lo:], start=True, stop=True)
                pt = ptpool.tile([P, IC], BF16, tag="pt", name="pt")
                nc.scalar.activation(
                    pt[:, lo:], ps_s[:, lo:],
                    mybir.ActivationFunctionType.Exp, scale=scale)
                if m >= 0:
                    nc.vector.tensor_tensor(
                        pt[:, lo:lo + P], pt[:, lo:lo + P], tri,
                        mybir.AluOpType.mult)
                if jt == 0:
                    nc.vector.tensor_copy(pt_acc, pt)
                else:
                    nc.vector.tensor_tensor(
                        pt_acc[:, lo:], pt_acc[:, lo:], pt[:, lo:],
                        mybir.AluOpType.add)
                nc.tensor.matmul(
                    ps_o[:, lo:], v_sb[h][b][:, jt], pt[:, lo:],
                    start=(jt == 0), stop=(jt == nj - 1),
                    skip_group_check=True)
            # denominator: one partition-sum matmul on the accumulated probs
            ps_d = psum.tile([1, IC], F32, tag="den", name="den")
            nc.tensor.matmul(ps_d, ones_col, pt_acc, start=True, stop=True,
                             skip_group_check=True)
            den_sb = smalls.tile([1, IC], F32, tag="densb", name="densb")
            nc.scalar.copy(den_sb, ps_d)
            rb = smalls.tile([1, IC], F32, tag="rb", name="rb")
            nc.vector.reciprocal_approx_fast(out=rb, in_=den_sb)
            bc = smalls.tile([P, IC], F32, tag="bc", name="bc")
            nc.gpsimd.partition_broadcast(bc, rb)
            nc.vector.tensor_tensor(
                outT[h][b][:, qoff:qoff + IC], ps_o, bc, mybir.AluOpType.mult)

        # ---- out projection for one 128-token tile ----
        def outproj_tt(tt, deep=False):
            b, jt = tt // TJ, tt % TJ
            tags = (("pq", "pk", "pv", "o0", "o1", "s0")
                    if deep else ("pq", "pk", "pv"))
            y_sb = ypool.tile([P, DM], cfg["y"], tag="ysb", name="ysb")
            for mc in range(4):
                msl = slice(mc * IC, (mc + 1) * IC)
                psy = psum.tile([P, IC], F32,
                                tag=tags[(tt * 4 + mc) % len(tags)],
                                name="py")
                for h in range(HPC):
                    nc.tensor.matmul(
                        psy, outT[h][b][:, jt * P:(jt + 1) * P],
                        wo_sb[h][:, msl], start=(h == 0), stop=(h == HPC - 1))
                copy_out(y_sb[:, msl], psy)
            nc.sync.dma_start(y[tt * P:(tt + 1) * P, :], y_sb)

        # ---- emission: batch-pipelined ----
        xt0 = xpool.tile([P, KD, IC], BF16, tag="xt", name="xt")
        # split the first x chunk's DMA so the first matmuls start sooner
        nc.sync.dma_start(xt0[:, :KD // 2], xT4[:, 0, :KD // 2])
        nc.sync.dma_start(wq_sb[0], wq[0])
        nc.sync.dma_start(xt0[:, KD // 2:], xT4[:, 0, KD // 2:])
        nc.sync.dma_start(wk_sb[0], wk[0])
        nc.sync.dma_start(wv_sb[0], wv[0])
        xt8_0 = None
        if p8:
            xt8_0 = xpool.tile([P, KD, IC], FP8, tag="xt8", name="xt8")
            nc.sync.dma_start(xt8_0, xT8[:, 0])
        load_w(1)
        nc.sync.dma_start(wo_sb[0], wo[0])
        nc.sync.dma_start(wo_sb[1], wo[1])
        proj_tch(0, xt=xt0, xt8=xt8_0)
        for tch in range(1, 4):                   # proj b0
            proj_tch(tch)
        attn_chunk(0, 0, 0)                       # proj b1 || attn b0
        attn_chunk(1, 0, 0)
        proj_tch(4)
        attn_chunk(0, 0, 1)
        attn_chunk(1, 0, 1)
        proj_tch(5)
        attn_chunk(0, 0, 2)
        attn_chunk(1, 0, 2)
        proj_tch(6)
        attn_chunk(0, 0, 3)
        proj_tch(7)
        attn_chunk(1, 0, 3)
        for ic in range(NI):                      # attn b1 || outproj b0
            attn_chunk(0, 1, ic)
            attn_chunk(1, 1, ic)
            for tt in range(ic * 4, ic * 4 + 4):
                outproj_tt(tt)
        for tt in range(TJ, 2 * TJ):              # outproj b1 (attn done:
            outproj_tt(tt, deep=True)             # deep psum rotation)

        if dbg is not None:
            for h in range(HPC):
                nc.sync.dma_start(dbg["wo"][h], wo_sb[h])
                for b in range(B):
                    nc.sync.dma_start(dbg["qT"][h, b], qT[h][b])
                    nc.sync.dma_start(dbg["kT"][h, b], kT[h][b])
                    nc.sync.dma_start(dbg["v"][h, b], v_sb[h][b])
                    nc.sync.dma_start(dbg["outT"][h, b], outT[h][b])


def _build(cfg):
    nc = bacc.Bacc("TRN2", target_bir_lowering=False, debug=False,
                   num_devices=NCORES)
    qk_dt = FP8 if cfg["p8"] else BF16
    xT4 = nc.dram_tensor("xT4", [P, TCH, KD, IC], BF16,
                         kind="ExternalInput").ap()
    xT8 = None
    if cfg["p8"]:
        xT8 = nc.dram_tensor("xT8", [P, TCH, KD, IC], FP8,
                             kind="ExternalInput").ap()
    wq = nc.dram_tensor("wq", [HPC, P, KD, P], qk_dt,
                        kind="ExternalInput").ap()
    wk = nc.dram_tensor("wk", [HPC, P, KD, P], qk_dt,
                        kind="ExternalInput").ap()
    wv = nc.dram_tensor("wv", [HPC, P, KD, P], BF16, kind="ExternalInput").ap()
    wo = nc.dram_tensor("wo", [HPC, P, DM], BF16, kind="ExternalInput").ap()
    y = nc.dram_tensor("y", [BT, DM], cfg["y"], kind="ExternalOutput").ap()
    dbg = None
    if DEBUG_DUMPS:
        dbg = {
            "qT": nc.dram_tensor("dqT", [HPC, B, P, T], cfg["s"],
                                 kind="ExternalOutput").ap(),
            "kT": nc.dram_tensor("dkT", [HPC, B, P, T], cfg["s"],
                                 kind="ExternalOutput").ap(),
            "v": nc.dram_tensor("dv", [HPC, B, P, TJ, D], BF16,
                                kind="ExternalOutput").ap(),
            "outT": nc.dram_tensor("doutT", [HPC, B, P, T], cfg["o"],
                                   kind="ExternalOutput").ap(),
            "wo": nc.dram_tensor("dwo", [HPC, P, DM], BF16,
                                 kind="ExternalOutput").ap(),
        }
    with tile.TileContext(nc) as tc:
        _emit(tc, cfg, xT4, xT8, wq, wk, wv, wo, y, dbg)
    nc.compile()
    return nc


def _prep_inputs(x, Wq, Wk, Wv, Wo, cfg):
    bf = ml_dtypes.bfloat16
    f8 = ml_dtypes.float8_e4m3
    xt = np.asarray(x, np.float32).reshape(BT, DM)
    # [p, tch, kt, 512], contiguous per (p, tch)
    xT4f = np.ascontiguousarray(
        xt.reshape(TCH, IC, KD, P).transpose(3, 0, 2, 1))

    def wqkv(W, c, dt):
        Wc = np.asarray(W, np.float32)[c * HPC * D:(c + 1) * HPC * D]
        return np.ascontiguousarray(
            Wc.reshape(HPC, P, KD, P).transpose(0, 3, 2, 1)).astype(dt)

    qk_np = f8 if cfg["p8"] else bf
    in_maps = []
    for c in range(NCORES):
        Woc = np.asarray(Wo, np.float32)[:, c * HPC * D:(c + 1) * HPC * D]
        m = {
            "xT4": xT4f.astype(bf),
            "wq": wqkv(Wq, c, qk_np),
            "wk": wqkv(Wk, c, qk_np),
            "wv": wqkv(Wv, c, bf),
            "wo": np.ascontiguousarray(
                Woc.reshape(DM, HPC, P).transpose(1, 2, 0)).astype(bf),
        }
        if cfg["p8"]:
            m["xT8"] = xT4f.astype(f8)
        in_maps.append(m)
    return in_maps


def run(x, Wq, Wk, Wv, Wo, cfg=None, trace=False):
    cfg = cfg or CFG_FAST
    nc = _build(cfg)
    in_maps = _prep_inputs(x, Wq, Wk, Wv, Wo, cfg)
    try:
        res = run_bass_kernel_spmd(nc, in_maps, core_ids=list(range(NCORES)),
                                   trace=trace)
    except Exception:
        res = run_bass_kernel_spmd(nc, in_maps, core_ids=list(range(NCORES)),
                                   trace=trace)
    y = np.zeros((BT, DM), np.float32)
    for r in res.results:
        y += np.asarray(r["y"], np.float32)
    return y.reshape(B, T, DM), res


def kernel(x, Wq, Wk, Wv, Wo):
    y, _ = run(x, Wq, Wk, Wv, Wo)
    return y
